# revision 1
# baseline (speedup 1.0000x reference)
"""Trainium2 Bass kernel for nn_EncoderLayer_31825707664096.

Gemma-style encoder layer (RMSNorm + GQA attention w/ QK-norm + RoPE + GeGLU
MLP), batch=1, seq=2048, hidden=768, 3 heads x 256 head_dim, 1 KV head,
inter=1152, fp32.

Strategy: sequence-parallel over 8 cores (each core owns 256 query rows and
recomputes the full K/V — no collectives). All activations live feature-major
("transposed", [feature, seq]) in SBUF so no on-chip transposes are needed:

- weights are pre-transposed (and RMSNorm (1+w) pre-folded) on the host
- the input-norm rstd for Q/K is absorbed by q_norm/k_norm (both are
  scale-invariant per row); for V it rides the V PSUM->SBUF copy as a
  per-partition scale
- k_norm rstd is the per-partition `scale` of the softmax exp
- softmax runs WITHOUT max-subtraction or normalization: a constant shift
  exp(s - C) keeps fp32 in range, and the per-query normalizer is absorbed
  by the (scale-invariant) post-attention RMSNorm
- matmuls run in float32r (TF32-like, ~1.6e-4 rel err, full PE rate)

Per-core output is the feature-major [768, 256] shard; the host transposes
and concatenates.
"""

from contextlib import ExitStack

import numpy as np

import concourse.mybir as mybir
import concourse.tile as tile
from concourse import bacc
from concourse.bass_utils import run_bass_kernel_spmd

P = 128
S = 2048          # sequence length
H = 768           # hidden
D = 256           # head dim (also total KV width)
NH = 3            # query heads
I = 1152          # mlp intermediate
NC = 8            # cores
SL = S // NC      # 256 query rows per core
HC = H // P       # 6
DC = D // P       # 2
IC = I // P       # 9
KC = S // P       # 16 key chunks
NSL = S // 512    # 4 512-wide column slices
EPS = 1e-6
C_SHIFT = 30.0    # exp(s - C_SHIFT): keeps unnormalized softmax in fp32 range

F32 = mybir.dt.float32
F32R = mybir.dt.float32r
MUL = mybir.AluOpType.mult
AF = mybir.ActivationFunctionType

_CACHED = {}


def _build(debug=False):
    nc = bacc.Bacc("TRN2", target_bir_lowering=False, debug=False, num_devices=NC)

    # ---- DRAM I/O ----
    ht = nc.dram_tensor("ht", [H, S], F32R, kind="ExternalInput").ap()
    hq = nc.dram_tensor("hq", [H, SL], F32R, kind="ExternalInput").ap()
    cost = nc.dram_tensor("cost", [D, S], F32, kind="ExternalInput").ap()
    sint = nc.dram_tensor("sint", [D, S], F32, kind="ExternalInput").ap()
    cosq = nc.dram_tensor("cosq", [D, SL], F32, kind="ExternalInput").ap()
    sinq = nc.dram_tensor("sinq", [D, SL], F32, kind="ExternalInput").ap()
    wqt = nc.dram_tensor("wqt", [H, H], F32R, kind="ExternalInput").ap()
    wkt = nc.dram_tensor("wkt", [H, D], F32R, kind="ExternalInput").ap()
    wvt = nc.dram_tensor("wvt", [H, D], F32R, kind="ExternalInput").ap()
    wot = nc.dram_tensor("wot", [H, H], F32R, kind="ExternalInput").ap()
    wgt = nc.dram_tensor("wgt", [H, I], F32R, kind="ExternalInput").ap()
    wut = nc.dram_tensor("wut", [H, I], F32R, kind="ExternalInput").ap()
    wdt = nc.dram_tensor("wdt", [I, H], F32R, kind="ExternalInput").ap()
    qw1 = nc.dram_tensor("qw1", [P, DC], F32, kind="ExternalInput").ap()   # 1+q_norm_w
    kw1 = nc.dram_tensor("kw1", [P, DC], F32, kind="ExternalInput").ap()   # 1+k_norm_w
    waw = nc.dram_tensor("waw", [P, HC], F32, kind="ExternalInput").ap()   # 1+ln_post_attn_w
    wfw = nc.dram_tensor("wfw", [P, HC], F32, kind="ExternalInput").ap()   # 1+ln_post_ffn_w
    ones_in = nc.dram_tensor("ones_in", [P, 1], F32R, kind="ExternalInput").ap()
    outt = nc.dram_tensor("outt", [H, SL], F32, kind="ExternalOutput").ap()
    if debug:
        d_qt = nc.dram_tensor("d_qt", [P, HC, SL], F32, kind="ExternalOutput").ap()
        d_kt = nc.dram_tensor("d_kt", [P, DC, S], F32, kind="ExternalOutput").ap()
        d_v = nc.dram_tensor("d_v", [P, KC, D], F32, kind="ExternalOutput").ap()
        d_at = nc.dram_tensor("d_at", [P, HC, SL], F32, kind="ExternalOutput").ap()
        d_h2 = nc.dram_tensor("d_h2", [P, HC, SL], F32, kind="ExternalOutput").ap()
        d_act = nc.dram_tensor("d_act", [P, IC, SL], F32, kind="ExternalOutput").ap()
        d_rin = nc.dram_tensor("d_rin", [P, KC], F32, kind="ExternalOutput").ap()
        d_ck = nc.dram_tensor("d_ck", [P, KC], F32, kind="ExternalOutput").ap()

    def cp(ap2d):  # [(c p), x] -> [p, c, x]
        return ap2d.rearrange("(c p) x -> p c x", p=P)

    def f32(ap):
        return ap.bitcast(F32)

    with tile.TileContext(nc) as tc:
        with (
            tc.tile_pool(name="persist", bufs=1) as pp,
            tc.tile_pool(name="wp", bufs=6) as wp,
            tc.tile_pool(name="dr", bufs=1, space="DRAM") as dr,
        ):
            # ---- constants / small inputs ----
            ones = pp.tile([P, 1], F32R, tag="ones")
            nc.sync.dma_start(ones[:], ones_in)
            qw1_sb = pp.tile([P, DC], F32, tag="qw1")
            nc.sync.dma_start(qw1_sb[:], qw1)
            kw1_sb = pp.tile([P, DC], F32, tag="kw1")
            nc.sync.dma_start(kw1_sb[:], kw1)
            waw_sb = pp.tile([P, HC], F32, tag="waw")
            nc.sync.dma_start(waw_sb[:], waw)
            wfw_sb = pp.tile([P, HC], F32, tag="wfw")
            nc.sync.dma_start(wfw_sb[:], wfw)
            eps128 = pp.tile([P, 1], F32, tag="eps128")
            nc.vector.memset(eps128[:], EPS)
            biasC = pp.tile([P, 1], F32, tag="biasC")
            nc.vector.memset(biasC[:], -C_SHIFT)

            hq_sb = pp.tile([P, HC, SL], F32R, tag="hq")
            nc.sync.dma_start(hq_sb[:], cp(hq))
            cosq_sb = pp.tile([P, DC, SL], F32, tag="cosq")
            nc.sync.dma_start(cosq_sb[:], cp(cosq))
            sinq_sb = pp.tile([P, DC, SL], F32, tag="sinq")
            nc.sync.dma_start(sinq_sb[:], cp(sinq))

            # persistent activations
            qt_f = pp.tile([P, HC, SL], F32R, tag="qtf")
            kt_f = pp.tile([P, DC, S], F32R, tag="ktf")
            v_sb = pp.tile([P, KC, D], F32R, tag="v")
            at_f = pp.tile([P, HC, SL], F32R, tag="atf")
            h2 = pp.tile([P, HC, SL], F32, tag="h2")
            h2n = pp.tile([P, HC, SL], F32R, tag="h2n")
            rin_col = pp.tile([P, KC], F32, tag="rin")
            ck_col = pp.tile([P, KC], F32, tag="ck")

            scr = dr.tile([1, 2 * S], F32)

            # ---------- phase 0-2 pools (freed before attention) ----------
            es = ExitStack()
            htp = es.enter_context(tc.tile_pool(name="htp", bufs=1))
            csp = es.enter_context(tc.tile_pool(name="csp", bufs=2))
            t1 = es.enter_context(tc.tile_pool(name="t1", bufs=2))
            pmm = es.enter_context(tc.tile_pool(name="pmmA", bufs=3, space="PSUM"))
            pst = es.enter_context(tc.tile_pool(name="pstA", bufs=1, space="PSUM"))

            ht_sb = htp.tile([P, HC, S], F32R, tag="ht")
            for kc in range(HC):
                nc.sync.dma_start(ht_sb[:, kc, :], cp(ht)[:, kc, :])

            # =====================================================
            # Q projection + q-norm stats + RoPE (own 256 columns)
            # =====================================================
            wq_ch = []
            for kc in range(HC):
                w = wp.tile([P, I], F32R, tag="w", name=f"wq{kc}")
                nc.sync.dma_start(w[:, :H], cp(wqt)[:, kc, :])
                wq_ch.append(w)

            for h in range(NH):
                pq = [pmm.tile([P, SL], F32, tag="mm", name=f"pq{h}_{d_}")
                      for d_ in range(DC)]
                for d in range(DC):
                    oc = 2 * h + d
                    for kc in range(HC):
                        nc.tensor.matmul(
                            pq[d][:],
                            wq_ch[kc][:, oc * P:(oc + 1) * P],
                            hq_sb[:, kc, :],
                            start=(kc == 0), stop=(kc == HC - 1),
                        )
                # raw-q squares -> sumsq over d -> cq = rsqrt(mean+eps), bcast
                qss = pst.tile([1, SL], F32, tag="st1", name=f"qss{h}")
                for d in range(DC):
                    sq = t1.tile([P, SL], F32R, tag="sq256", name=f"qsq{h}_{d}")
                    nc.scalar.activation(sq[:], pq[d][:], AF.Square)
                    nc.tensor.matmul(qss[:], ones[:], sq[:],
                                     start=(d == 0), stop=(d == DC - 1))
                qss_sb = t1.tile([1, SL], F32, tag="row256", name=f"qssr{h}")
                nc.scalar.copy(qss_sb[:], qss[:])
                cq_b = t1.tile([P, SL], F32, tag="cqb", name=f"cqb{h}")
                nc.gpsimd.partition_broadcast(cq_b[:], qss_sb[:], channels=P)
                nc.scalar.activation(cq_b[:], cq_b[:], AF.Sqrt,
                                     bias=eps128[:], scale=1.0 / D)
                nc.vector.reciprocal(cq_b[:], cq_b[:])
                # rope + cq + (1+qw)
                t0 = t1.tile([P, SL], F32, tag="ropeA", name=f"rA{h}")
                tb = t1.tile([P, SL], F32, tag="ropeB", name=f"rB{h}")
                nc.vector.scalar_tensor_tensor(
                    t0[:], pq[0][:], qw1_sb[:, 0:1], cosq_sb[:, 0, :], MUL, MUL)
                nc.vector.scalar_tensor_tensor(
                    tb[:], pq[1][:], qw1_sb[:, 1:2], sinq_sb[:, 0, :], MUL, MUL)
                nc.vector.tensor_sub(t0[:], t0[:], tb[:])
                nc.vector.tensor_mul(qt_f[:, 2 * h, :], t0[:], cq_b[:])
                t2 = t1.tile([P, SL], F32, tag="ropeA", name=f"rC{h}")
                t3 = t1.tile([P, SL], F32, tag="ropeB", name=f"rD{h}")
                nc.vector.scalar_tensor_tensor(
                    t2[:], pq[1][:], qw1_sb[:, 1:2], cosq_sb[:, 1, :], MUL, MUL)
                nc.vector.scalar_tensor_tensor(
                    t3[:], pq[0][:], qw1_sb[:, 0:1], sinq_sb[:, 1, :], MUL, MUL)
                nc.vector.tensor_add(t2[:], t2[:], t3[:])
                nc.vector.tensor_mul(qt_f[:, 2 * h + 1, :], t2[:], cq_b[:])

            # =====================================================
            # input-norm sumsq over full S (for the V scale)
            # =====================================================
            iss = pst.tile([1, NSL, 512], F32, tag="st4", name="iss")
            for kc in range(HC):
                for sl in range(NSL):
                    sl_s = slice(sl * 512, (sl + 1) * 512)
                    sq = t1.tile([P, 512], F32R, tag="sq512", name=f"isq{kc}_{sl}")
                    nc.vector.tensor_mul(sq[:], f32(ht_sb[:, kc, sl_s]),
                                         f32(ht_sb[:, kc, sl_s]))
                    nc.tensor.matmul(iss[:, sl, :], ones[:], sq[:],
                                     start=(kc == 0), stop=(kc == HC - 1))
            iss_sb = t1.tile([1, S], F32, tag="row2048", name="iss_sb", bufs=1)
            nc.scalar.copy(iss_sb[:], iss[:].rearrange("o a b -> o (a b)"))
            nc.sync.dma_start(scr[0:1, 0:S], iss_sb[:])

            # =====================================================
            # K projection + k-norm stats + RoPE (full S)
            # =====================================================
            wk_ch = []
            for kc in range(HC):
                w = wp.tile([P, I], F32R, tag="w", name=f"wk{kc}")
                nc.sync.dma_start(w[:, :D], cp(wkt)[:, kc, :])
                wk_ch.append(w)

            kss = pst.tile([1, NSL, 512], F32, tag="st4", name="kss")
            for sl in range(NSL):
                sl_s = slice(sl * 512, (sl + 1) * 512)
                cs = csp.tile([P, DC, 512], F32, tag="cos512", name=f"cos{sl}")
                nc.sync.dma_start(cs[:], cp(cost)[:, :, sl_s])
                sn = csp.tile([P, DC, 512], F32, tag="sin512", name=f"sin{sl}")
                nc.sync.dma_start(sn[:], cp(sint)[:, :, sl_s])
                pk = [pmm.tile([P, 512], F32, tag="mm", name=f"pk{sl}_{d_}")
                      for d_ in range(DC)]
                for d in range(DC):
                    for kc in range(HC):
                        nc.tensor.matmul(
                            pk[d][:],
                            wk_ch[kc][:, d * P:(d + 1) * P],
                            ht_sb[:, kc, sl_s],
                            start=(kc == 0), stop=(kc == HC - 1),
                        )
                for d in range(DC):
                    sq = t1.tile([P, 512], F32R, tag="sq512", name=f"ksq{sl}_{d}")
                    nc.scalar.activation(sq[:], pk[d][:], AF.Square)
                    nc.tensor.matmul(kss[:, sl, :], ones[:], sq[:],
                                     start=(d == 0), stop=(d == DC - 1))
                t0 = t1.tile([P, 512], F32, tag="kropeA", name=f"krA{sl}")
                tb = t1.tile([P, 512], F32, tag="kropeB", name=f"krB{sl}")
                nc.vector.scalar_tensor_tensor(
                    t0[:], pk[0][:], kw1_sb[:, 0:1], cs[:, 0, :], MUL, MUL)
                nc.vector.scalar_tensor_tensor(
                    tb[:], pk[1][:], kw1_sb[:, 1:2], sn[:, 0, :], MUL, MUL)
                nc.vector.tensor_sub(kt_f[:, 0, sl_s], t0[:], tb[:])
                t2 = t1.tile([P, 512], F32, tag="kropeA", name=f"krC{sl}")
                t3 = t1.tile([P, 512], F32, tag="kropeB", name=f"krD{sl}")
                nc.vector.scalar_tensor_tensor(
                    t2[:], pk[1][:], kw1_sb[:, 1:2], cs[:, 1, :], MUL, MUL)
                nc.vector.scalar_tensor_tensor(
                    t3[:], pk[0][:], kw1_sb[:, 0:1], sn[:, 1, :], MUL, MUL)
                nc.vector.tensor_add(kt_f[:, 1, sl_s], t2[:], t3[:])
            kss_sb = t1.tile([1, S], F32, tag="row2048", name="kss_sb", bufs=1)
            nc.scalar.copy(kss_sb[:], kss[:].rearrange("o a b -> o (a b)"))
            nc.sync.dma_start(scr[0:1, S:2 * S], kss_sb[:])

            # stat vectors -> key-partition-major [P, 16]; rstd lane-parallel
            with nc.allow_non_contiguous_dma(reason="stat vector transpose"):
                nc.sync.dma_start(
                    rin_col[:], scr[0:1, 0:S].rearrange("o (c p) -> (o p) c", p=P))
                nc.sync.dma_start(
                    ck_col[:], scr[0:1, S:2 * S].rearrange("o (c p) -> (o p) c", p=P))
            nc.scalar.activation(rin_col[:], rin_col[:], AF.Sqrt,
                                 bias=eps128[:], scale=1.0 / H)
            nc.vector.reciprocal(rin_col[:], rin_col[:])
            nc.scalar.activation(ck_col[:], ck_col[:], AF.Sqrt,
                                 bias=eps128[:], scale=1.0 / D)
            nc.vector.reciprocal(ck_col[:], ck_col[:])

            # =====================================================
            # V projection (full S, natural [s, d] layout)
            # =====================================================
            wv_ch = []
            for kc in range(HC):
                w = wp.tile([P, I], F32R, tag="w", name=f"wv{kc}")
                nc.sync.dma_start(w[:, :D], cp(wvt)[:, kc, :])
                wv_ch.append(w)

            for sc in range(KC):
                pv = pmm.tile([P, D], F32, tag="mm", name=f"pv{sc}")
                for kc in range(HC):
                    nc.tensor.matmul(
                        pv[:],
                        ht_sb[:, kc, sc * P:(sc + 1) * P],
                        wv_ch[kc][:, :D],
                        start=(kc == 0), stop=(kc == HC - 1),
                    )
                nc.scalar.mul(v_sb[:, sc, :], pv[:], rin_col[:, sc:sc + 1])

            es.close()  # free ht / cos/sin / t1 temps

            # ---------- attention/ffn-phase pool ----------
            with tc.tile_pool(name="t2", bufs=2) as t2p:
                es2 = ExitStack()
                pmm = es2.enter_context(
                    tc.tile_pool(name="pmmB", bufs=3, space="PSUM"))
                pst = es2.enter_context(
                    tc.tile_pool(name="pstB", bufs=1, space="PSUM"))
                # =====================================================
                # attention per head: scoresT -> exp -> num^T (A^T)
                # =====================================================
                for h in range(NH):
                    u_sb = t2p.tile([P, KC, SL], F32R, tag="u", name=f"u{h}")
                    for ksc in range(KC):
                        ps_ = pmm.tile([P, SL], F32, tag="mm", name=f"ps{h}_{ksc}")
                        for d in range(DC):
                            nc.tensor.matmul(
                                ps_[:],
                                kt_f[:, d, ksc * P:(ksc + 1) * P],
                                qt_f[:, 2 * h + d, :],
                                start=(d == 0), stop=(d == DC - 1),
                            )
                        nc.scalar.activation(u_sb[:, ksc, :], ps_[:], AF.Exp,
                                             bias=biasC[:],
                                             scale=ck_col[:, ksc:ksc + 1])
                    # per-head softmax denominator, then A^T = num^T / den
                    den = pst.tile([1, SL], F32, tag="st1", name=f"den{h}")
                    for ksc in range(KC):
                        nc.tensor.matmul(den[:], ones[:], u_sb[:, ksc, :],
                                         start=(ksc == 0), stop=(ksc == KC - 1))
                    den_sb = t2p.tile([1, SL], F32, tag="row256b", name=f"den_sb{h}")
                    nc.scalar.copy(den_sb[:], den[:])
                    den_b = t2p.tile([P, SL], F32, tag="rstdb", name=f"den_b{h}")
                    nc.gpsimd.partition_broadcast(den_b[:], den_sb[:], channels=P)
                    nc.vector.reciprocal(den_b[:], den_b[:])
                    for d in range(DC):
                        pn = pmm.tile([P, SL], F32, tag="mm", name=f"pn{h}_{d}")
                        for ksc in range(KC):
                            nc.tensor.matmul(
                                pn[:],
                                v_sb[:, ksc, d * P:(d + 1) * P],
                                u_sb[:, ksc, :],
                                start=(ksc == 0), stop=(ksc == KC - 1),
                            )
                        nc.vector.tensor_mul(at_f[:, 2 * h + d, :], pn[:], den_b[:])

                # =====================================================
                # wo projection + post-attn rmsnorm + residual
                # =====================================================
                wo_ch = []
                for oc in range(HC):
                    w = wp.tile([P, I], F32R, tag="w", name=f"wo{oc}")
                    nc.sync.dma_start(w[:, :H], cp(wot)[:, oc, :])
                    wo_ch.append(w)

                proj = t2p.tile([P, HC, SL], F32, tag="big6", name="proj", bufs=2)
                pss = pst.tile([1, SL], F32, tag="st1", name="pss")
                for hc in range(HC):
                    pp_ = pmm.tile([P, SL], F32, tag="mm", name=f"pp{hc}")
                    for oc in range(HC):
                        nc.tensor.matmul(
                            pp_[:],
                            wo_ch[oc][:, hc * P:(hc + 1) * P],
                            at_f[:, oc, :],
                            start=(oc == 0), stop=(oc == HC - 1),
                        )
                    sq = t2p.tile([P, SL], F32R, tag="sq256b", name=f"psq{hc}")
                    nc.scalar.activation(sq[:], pp_[:], AF.Square)
                    nc.tensor.matmul(pss[:], ones[:], sq[:],
                                     start=(hc == 0), stop=(hc == HC - 1))
                    nc.scalar.mul(proj[:, hc, :], pp_[:], waw_sb[:, hc:hc + 1])
                pss_sb = t2p.tile([1, SL], F32, tag="row256b", name="pss_sb")
                nc.scalar.copy(pss_sb[:], pss[:])
                ra_b = t2p.tile([P, SL], F32, tag="rstdb", name="ra_b")
                nc.gpsimd.partition_broadcast(ra_b[:], pss_sb[:], channels=P)
                nc.scalar.activation(ra_b[:], ra_b[:], AF.Sqrt,
                                     bias=eps128[:], scale=1.0 / H)
                nc.vector.reciprocal(ra_b[:], ra_b[:])

                for hc in range(HC):
                    nc.vector.tensor_mul(proj[:, hc, :], proj[:, hc, :], ra_b[:])
                    nc.vector.tensor_add(h2[:, hc, :], proj[:, hc, :],
                                         f32(hq_sb[:, hc, :]))

                # =====================================================
                # pre-FFN rmsnorm
                # =====================================================
                fss = pst.tile([1, SL], F32, tag="st1", name="fss")
                for hc in range(HC):
                    sq = t2p.tile([P, SL], F32R, tag="sq256b", name=f"fsq{hc}")
                    nc.vector.tensor_mul(sq[:], h2[:, hc, :], h2[:, hc, :])
                    nc.tensor.matmul(fss[:], ones[:], sq[:],
                                     start=(hc == 0), stop=(hc == HC - 1))
                fss_sb = t2p.tile([1, SL], F32, tag="row256b", name="fss_sb")
                nc.scalar.copy(fss_sb[:], fss[:])
                r2_b = t2p.tile([P, SL], F32, tag="rstdb", name="r2_b")
                nc.gpsimd.partition_broadcast(r2_b[:], fss_sb[:], channels=P)
                nc.scalar.activation(r2_b[:], r2_b[:], AF.Sqrt,
                                     bias=eps128[:], scale=1.0 / H)
                nc.vector.reciprocal(r2_b[:], r2_b[:])
                for hc in range(HC):
                    nc.vector.tensor_mul(h2n[:, hc, :], h2[:, hc, :], r2_b[:])

                # =====================================================
                # MLP: gate/up -> gelu_tanh * up -> down + post-ffn norm
                # =====================================================
                gall = t2p.tile([P, IC, SL], F32, tag="gall", name="gall", bufs=1)
                wg_ch = []
                for kc in range(HC):
                    w = wp.tile([P, I], F32R, tag="w", name=f"wg{kc}")
                    nc.sync.dma_start(w[:], cp(wgt)[:, kc, :])
                    wg_ch.append(w)
                for ic in range(IC):
                    pg = pmm.tile([P, SL], F32, tag="mm", name=f"pg{ic}")
                    for kc in range(HC):
                        nc.tensor.matmul(pg[:], wg_ch[kc][:, ic * P:(ic + 1) * P],
                                         h2n[:, kc, :],
                                         start=(kc == 0), stop=(kc == HC - 1))
                    nc.scalar.activation(gall[:, ic, :], pg[:], AF.Gelu_apprx_tanh)

                act = t2p.tile([P, IC, SL], F32R, tag="act", name="act", bufs=1)
                wu_ch = []
                for kc in range(HC):
                    w2 = wp.tile([P, I], F32R, tag="w", name=f"wu{kc}")
                    nc.sync.dma_start(w2[:], cp(wut)[:, kc, :])
                    wu_ch.append(w2)
                for ic in range(IC):
                    pu = pmm.tile([P, SL], F32, tag="mm", name=f"pu{ic}")
                    for kc in range(HC):
                        nc.tensor.matmul(pu[:], wu_ch[kc][:, ic * P:(ic + 1) * P],
                                         h2n[:, kc, :],
                                         start=(kc == 0), stop=(kc == HC - 1))
                    nc.vector.tensor_mul(act[:, ic, :], gall[:, ic, :], pu[:])

                mlp = t2p.tile([P, HC, SL], F32, tag="big6", name="mlp", bufs=2)
                es2.close()
                with tc.tile_pool(name="pmmD", bufs=1, space="PSUM") as pmmd, \
                     tc.tile_pool(name="pstD", bufs=1, space="PSUM") as pst:
                    pm = [pmmd.tile([P, SL], F32, tag=f"mmd{hc_}", name=f"pm{hc_}")
                          for hc_ in range(HC)]
                    for ic in range(IC):
                        w = wp.tile([P, I], F32R, tag="w", name=f"wd{ic}")
                        nc.sync.dma_start(w[:, :H], cp(wdt)[:, ic, :])
                        for hc in range(HC):
                            nc.tensor.matmul(pm[hc][:], w[:, hc * P:(hc + 1) * P],
                                             act[:, ic, :],
                                             start=(ic == 0), stop=(ic == IC - 1))
                    mss = pst.tile([1, SL], F32, tag="st1", name="mss")
                    for hc in range(HC):
                        sq = t2p.tile([P, SL], F32R, tag="sq256b", name=f"msq{hc}")
                        nc.scalar.activation(sq[:], pm[hc][:], AF.Square)
                        nc.tensor.matmul(mss[:], ones[:], sq[:],
                                         start=(hc == 0), stop=(hc == HC - 1))
                        nc.scalar.mul(mlp[:, hc, :], pm[hc][:], wfw_sb[:, hc:hc + 1])
                mss_sb = t2p.tile([1, SL], F32, tag="row256b", name="mss_sb")
                nc.scalar.copy(mss_sb[:], mss[:])
                r3_b = t2p.tile([P, SL], F32, tag="rstdb", name="r3_b")
                nc.gpsimd.partition_broadcast(r3_b[:], mss_sb[:], channels=P)
                nc.scalar.activation(r3_b[:], r3_b[:], AF.Sqrt,
                                     bias=eps128[:], scale=1.0 / H)
                nc.vector.reciprocal(r3_b[:], r3_b[:])

                for hc in range(HC):
                    nc.vector.tensor_mul(mlp[:, hc, :], mlp[:, hc, :], r3_b[:])
                    nc.vector.tensor_add(mlp[:, hc, :], mlp[:, hc, :], h2[:, hc, :])
                    nc.sync.dma_start(cp(outt)[:, hc, :], mlp[:, hc, :])
                if debug:
                    nc.sync.dma_start(d_qt, f32(qt_f[:]))
                    nc.sync.dma_start(d_kt, f32(kt_f[:]))
                    nc.sync.dma_start(d_v, f32(v_sb[:]))
                    nc.sync.dma_start(d_at, f32(at_f[:]))
                    nc.sync.dma_start(d_h2, h2[:])
                    nc.sync.dma_start(d_act, f32(act[:]))
                    nc.sync.dma_start(d_rin, rin_col[:])
                    nc.sync.dma_start(d_ck, ck_col[:])

    nc.compile()
    return nc


def _get_nc():
    if "nc" not in _CACHED:
        _CACHED["nc"] = _build()
    return _CACHED["nc"]


def _prep_inputs(hidden_states, cos, sin, wq, wk, wv, wo, q_norm_w, k_norm_w,
                 ln_in_w, ln_post_attn_w, ln_pre_ffn_w, ln_post_ffn_w,
                 wg, wu, wd):
    f = np.float32
    ct = np.ascontiguousarray

    hid = np.asarray(hidden_states, f)[0]            # [S, H]
    hT = ct(hid.T)                                   # [H, S]
    cosT = ct(np.asarray(cos, f)[0, 0].T)            # [D, S]
    sinT = ct(np.asarray(sin, f)[0, 0].T)

    g_in = 1.0 + np.asarray(ln_in_w, f)
    g_ffn = 1.0 + np.asarray(ln_pre_ffn_w, f)

    shared = {
        "ht": hT,
        "cost": cosT,
        "sint": sinT,
        "wqt": ct((np.asarray(wq, f) * g_in[None, :]).T),
        "wkt": ct((np.asarray(wk, f) * g_in[None, :]).T),
        "wvt": ct((np.asarray(wv, f) * g_in[None, :]).T),
        "wot": ct(np.asarray(wo, f).T),
        "wgt": ct((np.asarray(wg, f) * g_ffn[None, :]).T),
        "wut": ct((np.asarray(wu, f) * g_ffn[None, :]).T),
        "wdt": ct(np.asarray(wd, f).T),
        "qw1": ct((1.0 + np.asarray(q_norm_w, f)).reshape(DC, P).T),
        "kw1": ct((1.0 + np.asarray(k_norm_w, f)).reshape(DC, P).T),
        "waw": ct((1.0 + np.asarray(ln_post_attn_w, f)).reshape(HC, P).T),
        "wfw": ct((1.0 + np.asarray(ln_post_ffn_w, f)).reshape(HC, P).T),
        "ones_in": np.ones((P, 1), f),
    }
    in_maps = []
    for c in range(NC):
        cols = slice(c * SL, (c + 1) * SL)
        m = dict(shared)
        m["hq"] = ct(hT[:, cols])
        m["cosq"] = ct(cosT[:, cols])
        m["sinq"] = ct(sinT[:, cols])
        in_maps.append(m)
    return in_maps


def run(trace=False, tmpdir=None, **inputs):
    """Build (cached), run on 8 cores, reassemble. Returns (output, results)."""
    nc = _get_nc()
    in_maps = _prep_inputs(
        inputs["hidden_states"], inputs["cos"], inputs["sin"],
        inputs["wq"], inputs["wk"], inputs["wv"], inputs["wo"],
        inputs["q_norm_w"], inputs["k_norm_w"],
        inputs["ln_in_w"], inputs["ln_post_attn_w"],
        inputs["ln_pre_ffn_w"], inputs["ln_post_ffn_w"],
        inputs["wg"], inputs["wu"], inputs["wd"],
    )
    res = run_bass_kernel_spmd(nc, in_maps, list(range(NC)),
                               trace=trace, tmpdir=tmpdir)
    out = np.empty((S, H), np.float32)
    for c in range(NC):
        out[c * SL:(c + 1) * SL, :] = res.results[c]["outt"].T
    return out[None], res


def kernel(**inputs):
    out, _ = run(trace=False, **inputs)
    return out



# revision 12
# speedup vs baseline: 1.0787x; 1.0787x over previous
"""Trainium2 Bass kernel for nn_EncoderLayer_31825707664096.

Gemma-style encoder layer (RMSNorm + GQA attention w/ QK-norm + RoPE + GeGLU
MLP), batch=1, seq=2048, hidden=768, 3 heads x 256 head_dim, 1 KV head,
inter=1152, fp32.

Strategy: sequence-parallel over 8 cores, no collectives (cross-core sync
costs ~90us of launch skew here). Each core owns 256 query rows and
recomputes full K/V, streaming the hidden state through SBUF in 512-column
slices so compute starts ~3us in. All activations are feature-major.

Precision tiers: q/k score path float32r (TF32-like, full PE rate at
free>=256); exp output u and V are bf16 (linear averaging errors only);
O-projection and MLP run bf16 weights/activations. PSUM always fp32;
residual path fp32.

Folds (host or on-chip, all exact):
- (1+ln_in_w) into wq/wk/wv rows; (1+ln_pre_ffn_w) into wg/wu rows
- (1+ln_post_attn_w) into wo columns; the rmsnorm stats then reduce with
  1/(1+w)^2 weights so the rstd sees unscaled values. Same for wd.
- input-norm rstd (rin) cancels in q/k norm; for V it rides the PSUM->SBUF
  copy as a per-partition scale (column stats via transposed PE reduction)
- k-norm rstd (ck) is the per-partition scale of the softmax exp
- softmax runs unnormalized with a constant shift exp(s - C); the per-query
  normalizer is applied to the attention output (reciprocal + broadcast)
"""

from contextlib import ExitStack

import numpy as np
import ml_dtypes

import concourse.mybir as mybir
import concourse.tile as tile
from concourse import bacc
from concourse.bass_utils import run_bass_kernel_spmd

P = 128
S = 2048          # sequence length
H = 768           # hidden
D = 256           # head dim (also total KV width)
NH = 3            # query heads
I = 1152          # mlp intermediate
NC = 8            # cores
SL = S // NC      # 256 query rows per core
SC = SL // P      # 2
HC = H // P       # 6
DC = D // P       # 2
IC = I // P       # 9
KC = S // P       # 16 key chunks
NSL = S // 512    # 4 512-wide slices
EPS = 1e-6
C_SHIFT = 30.0    # exp(s - C_SHIFT): keeps unnormalized softmax in fp32 range

F32 = mybir.dt.float32
F32R = mybir.dt.float32r
BF16 = mybir.dt.bfloat16
MUL = mybir.AluOpType.mult
AF = mybir.ActivationFunctionType

_CACHED = {}


def _build():
    nc = bacc.Bacc("TRN2", target_bir_lowering=False, debug=False, num_devices=NC)

    # ---- DRAM I/O ----
    ht = nc.dram_tensor("ht", [H, S], F32R, kind="ExternalInput").ap()
    hq = nc.dram_tensor("hq", [H, SL], F32R, kind="ExternalInput").ap()
    cost = nc.dram_tensor("cost", [D, S], F32, kind="ExternalInput").ap()
    sint = nc.dram_tensor("sint", [D, S], F32, kind="ExternalInput").ap()
    cosq = nc.dram_tensor("cosq", [D, SL], F32, kind="ExternalInput").ap()
    sinq = nc.dram_tensor("sinq", [D, SL], F32, kind="ExternalInput").ap()
    wqt = nc.dram_tensor("wqt", [H, H], F32R, kind="ExternalInput").ap()
    wkt = nc.dram_tensor("wkt", [H, D], F32R, kind="ExternalInput").ap()
    wvt = nc.dram_tensor("wvt", [H, D], F32R, kind="ExternalInput").ap()
    wot = nc.dram_tensor("wot", [H, H], BF16, kind="ExternalInput").ap()
    wgt = nc.dram_tensor("wgt", [H, I], BF16, kind="ExternalInput").ap()
    wut = nc.dram_tensor("wut", [H, I], BF16, kind="ExternalInput").ap()
    wdt = nc.dram_tensor("wdt", [I, H], BF16, kind="ExternalInput").ap()
    qw1 = nc.dram_tensor("qw1", [P, DC], F32, kind="ExternalInput").ap()   # 1+q_norm_w
    kw1 = nc.dram_tensor("kw1", [P, DC], F32, kind="ExternalInput").ap()   # 1+k_norm_w
    wai = nc.dram_tensor("wai", [P, HC], BF16, kind="ExternalInput").ap()  # (1+ln_post_attn)^-2
    wfi = nc.dram_tensor("wfi", [P, HC], BF16, kind="ExternalInput").ap()  # (1+ln_post_ffn)^-2
    outt = nc.dram_tensor("outt", [H, SL], F32, kind="ExternalOutput").ap()

    def cp(ap2d):  # [(c p), x] -> [p, c, x]
        return ap2d.rearrange("(c p) x -> p c x", p=P)

    def f32(ap):
        return ap.bitcast(F32)

    with tile.TileContext(nc) as tc:
        with (
            tc.tile_pool(name="persist", bufs=1) as pp,
            tc.tile_pool(name="t1", bufs=2) as t1,
            tc.tile_pool(name="pmm", bufs=3, space="PSUM") as pmm,
            tc.tile_pool(name="pst", bufs=1, space="PSUM") as pst,
        ):
            # ---- small constants ----
            ones = pp.tile([P, 1], F32, tag="ones")
            nc.vector.memset(ones[:], 1.0)
            ones_r = ones[:].bitcast(F32R)
            ones_b = pp.tile([P, 1], BF16, tag="ones_b")
            nc.vector.memset(ones_b[:], 1.0)
            ones2b = pp.tile([P, 2], BF16, tag="ones2b")
            nc.vector.memset(ones2b[:], 1.0)
            eps128 = pp.tile([P, 1], F32, tag="eps128")
            nc.vector.memset(eps128[:], EPS)
            biasC = pp.tile([P, 1], F32, tag="biasC")
            nc.vector.memset(biasC[:], -C_SHIFT)
            qw1_sb = pp.tile([P, DC], F32, tag="qw1")
            nc.sync.dma_start(qw1_sb[:], qw1)
            kw1_sb = pp.tile([P, DC], F32, tag="kw1")
            nc.sync.dma_start(kw1_sb[:], kw1)
            wai_sb = pp.tile([P, HC], BF16, tag="wai")
            nc.sync.dma_start(wai_sb[:], wai)
            wfi_sb = pp.tile([P, HC], BF16, tag="wfi")
            nc.sync.dma_start(wfi_sb[:], wfi)

            # ---- startup-critical loads ----
            hq_sb = pp.tile([P, HC, SL], F32R, tag="hq")
            nc.sync.dma_start(hq_sb[:], cp(hq))
            es = ExitStack()
            wp1 = es.enter_context(tc.tile_pool(name="wp1", bufs=1))
            wk_sb = wp1.tile([P, HC, D], F32R, tag="wk")
            nc.sync.dma_start(wk_sb[:], cp(wkt))
            wv_sb = wp1.tile([P, HC, D], F32R, tag="wv")
            nc.sync.dma_start(wv_sb[:], cp(wvt))

            # persistent activations
            qt_f = pp.tile([P, HC, SL], F32R, tag="qtf")
            kt_sb = pp.tile([P, DC, S], F32R, tag="ktf")
            v_sb = pp.tile([P, KC, D], BF16, tag="v")
            at_f = pp.tile([P, HC, SL], BF16, tag="atf")
            h2 = pp.tile([P, HC, SL], F32, tag="h2")
            h2n = pp.tile([P, HC, SL], BF16, tag="h2n")
            rin_col = pp.tile([P, KC], F32, tag="rin")
            ck_col = pp.tile([P, KC], F32, tag="ck")

            # =====================================================
            # K + stats over full S, streamed in 512-col slices
            # =====================================================
            esl = ExitStack()
            slp = esl.enter_context(tc.tile_pool(name="slp", bufs=2))
            psc = esl.enter_context(tc.tile_pool(name="psc", bufs=1, space="PSUM"))
            iss_t = psc.tile([P, KC, 2], F32, tag="ips", name="iss")
            ck_t = psc.tile([P, KC, 2], F32, tag="cps", name="cks")
            iss_ps = iss_t[:]
            ck_ps = ck_t[:]

            for sl in range(NSL):
                sls = slice(sl * 512, (sl + 1) * 512)
                hsl = slp.tile([P, HC, 512], F32R, tag="hsl", name=f"hsl{sl}")
                nc.sync.dma_start(hsl[:], cp(ht)[:, :, sls])
                cs = slp.tile([P, DC, 512], F32, tag="cs", name=f"cs{sl}")
                nc.sync.dma_start(cs[:], cp(cost)[:, :, sls])
                sn = slp.tile([P, DC, 512], F32, tag="sn", name=f"sn{sl}")
                nc.sync.dma_start(sn[:], cp(sint)[:, :, sls])

                # K projection for this slice
                pk = [psc.tile([P, 512], F32, tag="pk", name=f"pk{sl}_{d_}", bufs=2)
                      for d_ in range(DC)]
                for d in range(DC):
                    for kc in range(HC):
                        nc.tensor.matmul(
                            pk[d][:],
                            wk_sb[:, kc, d * P:(d + 1) * P],
                            hsl[:, kc, :],
                            start=(kc == 0), stop=(kc == HC - 1),
                        )
                # k-norm stats (column orientation)
                ksq = []
                for d in range(DC):
                    sq = t1.tile([P, 512], BF16, tag="ksq", name=f"ksq{sl}_{d}")
                    nc.scalar.activation(sq[:], pk[d][:], AF.Square)
                    ksq.append(sq)
                for c4 in range(4):
                    ksc = 4 * sl + c4
                    for d in range(DC):
                        nc.tensor.matmul(
                            ck_ps[:, ksc, :],
                            ksq[d][:, c4 * P:(c4 + 1) * P],
                            ones2b[:],
                            start=(d == 0), stop=(d == DC - 1),
                        )
                # rope -> kt
                t0 = t1.tile([P, 512], F32, tag="rA", name=f"krA{sl}")
                tb = t1.tile([P, 512], F32, tag="rB", name=f"krB{sl}")
                nc.vector.scalar_tensor_tensor(
                    t0[:], pk[0][:], kw1_sb[:, 0:1], cs[:, 0, :], MUL, MUL)
                nc.vector.scalar_tensor_tensor(
                    tb[:], pk[1][:], kw1_sb[:, 1:2], sn[:, 0, :], MUL, MUL)
                nc.vector.tensor_sub(kt_sb[:, 0, sls], t0[:], tb[:])
                t2 = t1.tile([P, 512], F32, tag="rA", name=f"krC{sl}")
                t3 = t1.tile([P, 512], F32, tag="rB", name=f"krD{sl}")
                nc.vector.scalar_tensor_tensor(
                    t2[:], pk[1][:], kw1_sb[:, 1:2], cs[:, 1, :], MUL, MUL)
                nc.vector.scalar_tensor_tensor(
                    t3[:], pk[0][:], kw1_sb[:, 0:1], sn[:, 1, :], MUL, MUL)
                nc.vector.tensor_add(kt_sb[:, 1, sls], t2[:], t3[:])

                # input sumsq (column orientation) for V scale
                isq_t = []
                for kc in range(HC):
                    isq = t1.tile([P, 512], BF16, tag="isq", name=f"isq{sl}_{kc}",
                                  bufs=HC)
                    nc.vector.tensor_mul(isq[:], f32(hsl[:, kc, :]),
                                         f32(hsl[:, kc, :]))
                    isq_t.append(isq)
                for c4 in range(4):
                    for kc in range(HC):
                        nc.tensor.matmul(
                            iss_ps[:, 4 * sl + c4, :],
                            isq_t[kc][:, c4 * P:(c4 + 1) * P],
                            ones2b[:],
                            start=(kc == 0), stop=(kc == HC - 1),
                        )

                # V projection for this slice (raw; rin applied afterwards)
                for c4 in range(4):
                    sc = 4 * sl + c4
                    pv = pmm.tile([P, D], F32, tag="mm", name=f"pv{sc}")
                    for kc in range(HC):
                        nc.tensor.matmul(
                            pv[:],
                            hsl[:, kc, c4 * P:(c4 + 1) * P],
                            wv_sb[:, kc, :],
                            start=(kc == 0), stop=(kc == HC - 1),
                        )
                    nc.scalar.copy(v_sb[:, sc, :], pv[:])

            # rstd columns: rin [P, KC], ck [P, KC]
            nc.scalar.activation(
                rin_col[:],
                iss_ps[:, :, 0:1].rearrange("p a b -> p (a b)"),
                AF.Sqrt, bias=eps128[:], scale=1.0 / H)
            nc.vector.reciprocal(rin_col[:], rin_col[:])
            nc.scalar.activation(
                ck_col[:],
                ck_ps[:, :, 0:1].rearrange("p a b -> p (a b)"),
                AF.Sqrt, bias=eps128[:], scale=1.0 / D)
            nc.vector.reciprocal(ck_col[:], ck_col[:])

            # apply rin to V in place
            for sc in range(KC):
                nc.scalar.mul(v_sb[:, sc, :], v_sb[:, sc, :],
                              rin_col[:, sc:sc + 1])

            esl.close()  # free slice ring + slice psum

            # =====================================================
            # Q projection + q-norm + rope
            # =====================================================
            cosq_sb = pp.tile([P, DC, SL], F32, tag="cosq")
            nc.sync.dma_start(cosq_sb[:], cp(cosq))
            sinq_sb = pp.tile([P, DC, SL], F32, tag="sinq")
            nc.sync.dma_start(sinq_sb[:], cp(sinq))
            wq_sb = wp1.tile([P, HC, H], F32R, tag="wq")
            nc.sync.dma_start(wq_sb[:], cp(wqt))
            wo_sb = pp.tile([P, HC, H], BF16, tag="wo")
            nc.sync.dma_start(wo_sb[:], cp(wot))
            wg_sb = pp.tile([P, HC, I], BF16, tag="wg")
            nc.sync.dma_start(wg_sb[:], cp(wgt))
            wu_sb = pp.tile([P, HC, I], BF16, tag="wu")
            nc.sync.dma_start(wu_sb[:], cp(wut))
            wd_sb = pp.tile([P, IC, H], BF16, tag="wd")
            nc.sync.dma_start(wd_sb[:], cp(wdt))

            for h in range(NH):
                pq = [pmm.tile([P, SL], F32, tag="mm", name=f"pq{h}_{d_}")
                      for d_ in range(DC)]
                for d in range(DC):
                    oc = 2 * h + d
                    for kc in range(HC):
                        nc.tensor.matmul(
                            pq[d][:],
                            wq_sb[:, kc, oc * P:(oc + 1) * P],
                            hq_sb[:, kc, :],
                            start=(kc == 0), stop=(kc == HC - 1),
                        )
                qss = pst.tile([1, SL], F32, tag="st1", name=f"qss{h}")
                for d in range(DC):
                    sq = t1.tile([P, SL], F32R, tag="sq", name=f"qsq{h}_{d}")
                    nc.scalar.activation(sq[:], pq[d][:], AF.Square)
                    nc.tensor.matmul(qss[:], ones_r, sq[:],
                                     start=(d == 0), stop=(d == DC - 1))
                cq_row = t1.tile([1, SL], F32, tag="row", name=f"cqr{h}")
                nc.scalar.activation(cq_row[:], qss[:], AF.Sqrt,
                                     bias=eps128[0:1, :], scale=1.0 / D)
                nc.vector.reciprocal(cq_row[:], cq_row[:])
                cq_b = t1.tile([P, SL], F32, tag="bcast", name=f"cqb{h}")
                nc.gpsimd.partition_broadcast(cq_b[:], cq_row[:], channels=P)
                t0 = t1.tile([P, SL], F32, tag="rA", name=f"rA{h}")
                tb = t1.tile([P, SL], F32, tag="rB", name=f"rB{h}")
                nc.vector.scalar_tensor_tensor(
                    t0[:], pq[0][:], qw1_sb[:, 0:1], cosq_sb[:, 0, :], MUL, MUL)
                nc.vector.scalar_tensor_tensor(
                    tb[:], pq[1][:], qw1_sb[:, 1:2], sinq_sb[:, 0, :], MUL, MUL)
                nc.vector.tensor_sub(t0[:], t0[:], tb[:])
                nc.vector.tensor_mul(qt_f[:, 2 * h, :], t0[:], cq_b[:])
                t2 = t1.tile([P, SL], F32, tag="rA", name=f"rC{h}")
                t3 = t1.tile([P, SL], F32, tag="rB", name=f"rD{h}")
                nc.vector.scalar_tensor_tensor(
                    t2[:], pq[1][:], qw1_sb[:, 1:2], cosq_sb[:, 1, :], MUL, MUL)
                nc.vector.scalar_tensor_tensor(
                    t3[:], pq[0][:], qw1_sb[:, 0:1], sinq_sb[:, 1, :], MUL, MUL)
                nc.vector.tensor_add(t2[:], t2[:], t3[:])
                nc.vector.tensor_mul(qt_f[:, 2 * h + 1, :], t2[:], cq_b[:])

            es.close()  # free wq/wk/wv

            # =====================================================
            # attention: scoresT -> exp(scale=ck) -> den -> A^T V
            # =====================================================
            with tc.tile_pool(name="t2", bufs=2) as t2p:
                u3 = t2p.tile([P, NH, KC, SL], BF16, tag="u3", name="u3", bufs=1)
                den_b = t2p.tile([P, NH, SL], F32, tag="denb", name="den_b", bufs=1)
                for h in range(NH):
                    for ksc in range(KC):
                        ps_ = pmm.tile([P, SL], F32, tag="mm", name=f"ps{h}_{ksc}")
                        for d in range(DC):
                            nc.tensor.matmul(
                                ps_[:],
                                kt_sb[:, d, ksc * P:(ksc + 1) * P],
                                qt_f[:, 2 * h + d, :],
                                start=(d == 0), stop=(d == DC - 1),
                            )
                        nc.scalar.activation(u3[:, h, ksc, :], ps_[:], AF.Exp,
                                             bias=biasC[:],
                                             scale=ck_col[:, ksc:ksc + 1])
                for h in range(NH):
                    den = pst.tile([1, SL], F32, tag="st1", name=f"den{h}")
                    for ksc in range(KC):
                        nc.tensor.matmul(den[:], ones_b[:], u3[:, h, ksc, :],
                                         start=(ksc == 0), stop=(ksc == KC - 1))
                    den_row = t1.tile([1, SL], F32, tag="row", name=f"denr{h}")
                    nc.scalar.copy(den_row[:], den[:])
                    nc.vector.reciprocal(den_row[:], den_row[:])
                    nc.gpsimd.partition_broadcast(den_b[:, h, :], den_row[:],
                                                  channels=P)
                for h in range(NH):
                    for d in range(DC):
                        pn = pmm.tile([P, SL], F32, tag="mm", name=f"pn{h}_{d}")
                        for ksc in range(KC):
                            nc.tensor.matmul(
                                pn[:],
                                v_sb[:, ksc, d * P:(d + 1) * P],
                                u3[:, h, ksc, :],
                                start=(ksc == 0), stop=(ksc == KC - 1),
                            )
                        nc.vector.tensor_mul(at_f[:, 2 * h + d, :], pn[:],
                                             den_b[:, h, :])

                # =====================================================
                # wo projection + post-attn rmsnorm + residual
                # =====================================================
                with tc.tile_pool(name="pho", bufs=1, space="PSUM") as pho:
                    ppo2 = [pho.tile([P, 2, SL], F32, tag=f"po{j}", name=f"pp{j}")
                            for j in range(HC // 2)]
                    ppo = [ppo2[j][:, i, :] for j in range(HC // 2) for i in range(2)]
                    pss = pst.tile([1, SL], F32, tag="st1", name="pss")
                    for hc in range(HC):
                        for oc in range(HC):
                            nc.tensor.matmul(
                                ppo[hc],
                                wo_sb[:, oc, hc * P:(hc + 1) * P],
                                at_f[:, oc, :],
                                start=(oc == 0), stop=(oc == HC - 1),
                            )
                        sq = t1.tile([P, SL], BF16, tag="sqb", name=f"psq{hc}")
                        nc.scalar.activation(sq[:], ppo[hc], AF.Square)
                        nc.tensor.matmul(pss[:], wai_sb[:, hc:hc + 1], sq[:],
                                         start=(hc == 0), stop=(hc == HC - 1))
                    ra_row = t1.tile([1, SL], F32, tag="row", name="ra_row")
                    nc.scalar.activation(ra_row[:], pss[:], AF.Sqrt,
                                         bias=eps128[0:1, :], scale=1.0 / H)
                    nc.vector.reciprocal(ra_row[:], ra_row[:])
                    ra_b = t1.tile([P, SL], F32, tag="bcast", name="ra_b")
                    nc.gpsimd.partition_broadcast(ra_b[:], ra_row[:], channels=P)
                    for hc in range(HC):
                        tm = t1.tile([P, SL], F32, tag="htmp", name=f"hm{hc}")
                        nc.vector.tensor_mul(tm[:], ppo[hc], ra_b[:])
                        nc.vector.tensor_add(h2[:, hc, :], tm[:],
                                             f32(hq_sb[:, hc, :]))

                # =====================================================
                # pre-FFN rmsnorm
                # =====================================================
                fss = pst.tile([1, SL], F32, tag="st1", name="fss")
                for hc in range(HC):
                    sq = t1.tile([P, SL], BF16, tag="sqb", name=f"fsq{hc}")
                    nc.vector.tensor_mul(sq[:], h2[:, hc, :], h2[:, hc, :])
                    nc.tensor.matmul(fss[:], ones_b[:], sq[:],
                                     start=(hc == 0), stop=(hc == HC - 1))
                r2_row = t1.tile([1, SL], F32, tag="row", name="r2_row")
                nc.scalar.activation(r2_row[:], fss[:], AF.Sqrt,
                                     bias=eps128[0:1, :], scale=1.0 / H)
                nc.vector.reciprocal(r2_row[:], r2_row[:])
                r2_b = t1.tile([P, SL], F32, tag="bcast", name="r2_b")
                nc.gpsimd.partition_broadcast(r2_b[:], r2_row[:], channels=P)
                for hc in range(HC):
                    nc.vector.tensor_mul(h2n[:, hc, :], h2[:, hc, :], r2_b[:])

                # =====================================================
                # MLP: gate/up -> gelu_tanh * up -> down + post-ffn norm
                # =====================================================
                gall = t2p.tile([P, IC, SL], BF16, tag="gall", name="gall", bufs=1)
                act = t2p.tile([P, IC, SL], BF16, tag="act", name="act", bufs=1)
                for ic in range(IC):
                    pg = pmm.tile([P, SL], F32, tag="mm", name=f"pg{ic}")
                    for kc in range(HC):
                        nc.tensor.matmul(pg[:], wg_sb[:, kc, ic * P:(ic + 1) * P],
                                         h2n[:, kc, :],
                                         start=(kc == 0), stop=(kc == HC - 1))
                    nc.scalar.activation(gall[:, ic, :], pg[:], AF.Gelu_apprx_tanh)
                    pu = pmm.tile([P, SL], F32, tag="mm", name=f"pu{ic}")
                    for kc in range(HC):
                        nc.tensor.matmul(pu[:], wu_sb[:, kc, ic * P:(ic + 1) * P],
                                         h2n[:, kc, :],
                                         start=(kc == 0), stop=(kc == HC - 1))
                    nc.vector.tensor_mul(act[:, ic, :], gall[:, ic, :], pu[:])

                with tc.tile_pool(name="phd", bufs=1, space="PSUM") as phd:
                    pm2 = [phd.tile([P, 2, SL], F32, tag=f"md{j}", name=f"pm{j}")
                           for j in range(HC // 2)]
                    pm = [pm2[j][:, i, :] for j in range(HC // 2) for i in range(2)]
                    mss = pst.tile([1, SL], F32, tag="st1", name="mss")
                    for hc in range(HC):
                        for ic in range(IC):
                            nc.tensor.matmul(pm[hc],
                                             wd_sb[:, ic, hc * P:(hc + 1) * P],
                                             act[:, ic, :],
                                             start=(ic == 0), stop=(ic == IC - 1))
                        sq = t1.tile([P, SL], BF16, tag="sqb", name=f"msq{hc}")
                        nc.scalar.activation(sq[:], pm[hc], AF.Square)
                        nc.tensor.matmul(mss[:], wfi_sb[:, hc:hc + 1], sq[:],
                                         start=(hc == 0), stop=(hc == HC - 1))
                    r3_row = t1.tile([1, SL], F32, tag="row", name="r3_row")
                    nc.scalar.activation(r3_row[:], mss[:], AF.Sqrt,
                                         bias=eps128[0:1, :], scale=1.0 / H)
                    nc.vector.reciprocal(r3_row[:], r3_row[:])
                    r3_b = t1.tile([P, SL], F32, tag="bcast", name="r3_b")
                    nc.gpsimd.partition_broadcast(r3_b[:], r3_row[:], channels=P)
                    for hc in range(HC):
                        tm = t1.tile([P, SL], F32, tag="htmp", name=f"om{hc}")
                        nc.vector.tensor_mul(tm[:], pm[hc], r3_b[:])
                        out_c = t1.tile([P, SL], F32, tag="outc", name=f"oc{hc}",
                                        bufs=3)
                        nc.vector.tensor_add(out_c[:], tm[:], h2[:, hc, :])
                        nc.sync.dma_start(cp(outt)[:, hc, :], out_c[:])

    nc.compile()
    return nc


def _get_nc():
    if "nc" not in _CACHED:
        _CACHED["nc"] = _build()
    return _CACHED["nc"]


def _prep_inputs(hidden_states, cos, sin, wq, wk, wv, wo, q_norm_w, k_norm_w,
                 ln_in_w, ln_post_attn_w, ln_pre_ffn_w, ln_post_ffn_w,
                 wg, wu, wd):
    f = np.float32
    bf = ml_dtypes.bfloat16
    ct = np.ascontiguousarray

    hid = np.asarray(hidden_states, f)[0]            # [S, H]
    hT = ct(hid.T)                                   # [H, S]
    cosT = ct(np.asarray(cos, f)[0, 0].T)            # [D, S]
    sinT = ct(np.asarray(sin, f)[0, 0].T)

    g_in = 1.0 + np.asarray(ln_in_w, f)
    g_ffn = 1.0 + np.asarray(ln_pre_ffn_w, f)
    g_att = 1.0 + np.asarray(ln_post_attn_w, f)
    g_out = 1.0 + np.asarray(ln_post_ffn_w, f)

    shared = {
        "ht": hT,
        "cost": cosT,
        "sint": sinT,
        "wqt": ct((np.asarray(wq, f) * g_in[None, :]).T),
        "wkt": ct((np.asarray(wk, f) * g_in[None, :]).T),
        "wvt": ct((np.asarray(wv, f) * g_in[None, :]).T),
        "wot": ct((np.asarray(wo, f).T * g_att[None, :]).astype(bf)),
        "wgt": ct(((np.asarray(wg, f) * g_ffn[None, :]).T).astype(bf)),
        "wut": ct(((np.asarray(wu, f) * g_ffn[None, :]).T).astype(bf)),
        "wdt": ct((np.asarray(wd, f).T * g_out[None, :]).astype(bf)),
        "qw1": ct((1.0 + np.asarray(q_norm_w, f)).reshape(DC, P).T),
        "kw1": ct((1.0 + np.asarray(k_norm_w, f)).reshape(DC, P).T),
        "wai": ct((g_att ** -2.0).reshape(HC, P).T.astype(bf)),
        "wfi": ct((g_out ** -2.0).reshape(HC, P).T.astype(bf)),
    }
    in_maps = []
    for c in range(NC):
        cols = slice(c * SL, (c + 1) * SL)
        m = dict(shared)
        m["hq"] = ct(hT[:, cols])
        m["cosq"] = ct(cosT[:, cols])
        m["sinq"] = ct(sinT[:, cols])
        in_maps.append(m)
    return in_maps


def run(trace=False, tmpdir=None, **inputs):
    """Build (cached), run on 8 cores, reassemble. Returns (output, results)."""
    nc = _get_nc()
    in_maps = _prep_inputs(
        inputs["hidden_states"], inputs["cos"], inputs["sin"],
        inputs["wq"], inputs["wk"], inputs["wv"], inputs["wo"],
        inputs["q_norm_w"], inputs["k_norm_w"],
        inputs["ln_in_w"], inputs["ln_post_attn_w"],
        inputs["ln_pre_ffn_w"], inputs["ln_post_ffn_w"],
        inputs["wg"], inputs["wu"], inputs["wd"],
    )
    res = run_bass_kernel_spmd(nc, in_maps, list(range(NC)),
                               trace=trace, tmpdir=tmpdir)
    out = np.empty((S, H), np.float32)
    for c in range(NC):
        out[c * SL:(c + 1) * SL, :] = res.results[c]["outt"].T
    return out[None], res


def kernel(**inputs):
    out, _ = run(trace=False, **inputs)
    return out


# revision 13
# speedup vs baseline: 1.2197x; 1.1307x over previous
"""Trainium2 Bass kernel for nn_EncoderLayer_31825707664096.

Gemma-style encoder layer (RMSNorm + GQA attention w/ QK-norm + RoPE + GeGLU
MLP), batch=1, seq=2048, hidden=768, 3 heads x 256 head_dim, 1 KV head,
inter=1152, fp32.

Strategy: sequence-parallel over 8 cores, no collectives (cross-core sync
costs ~90us of launch skew here). Each core owns 256 query rows and
recomputes full K/V, streaming the hidden state through SBUF in 512-column
slices so compute starts ~3us in. All activations are feature-major.

Precision tiers: q/k score path float32r (TF32-like, full PE rate at
free>=256); exp output u and V are bf16 (linear averaging errors only);
O-projection and MLP run bf16 weights/activations. PSUM always fp32;
residual path fp32.

Folds (host or on-chip, all exact):
- (1+ln_in_w) into wq/wk/wv rows; (1+ln_pre_ffn_w) into wg/wu rows
- (1+ln_post_attn_w) into wo columns; the rmsnorm stats then reduce with
  1/(1+w)^2 weights so the rstd sees unscaled values. Same for wd.
- input-norm rstd (rin) cancels in q/k norm; for V it rides the PSUM->SBUF
  copy as a per-partition scale (column stats via transposed PE reduction)
- k-norm rstd (ck) is the per-partition scale of the softmax exp
- softmax runs unnormalized with a constant shift exp(s - C); the per-query
  normalizer is applied to the attention output (reciprocal + broadcast)
"""

from contextlib import ExitStack

import numpy as np
import ml_dtypes

import concourse.mybir as mybir
import concourse.tile as tile
from concourse import bacc
from concourse.bass_utils import run_bass_kernel_spmd

P = 128
S = 2048          # sequence length
H = 768           # hidden
D = 256           # head dim (also total KV width)
NH = 3            # query heads
I = 1152          # mlp intermediate
NC = 8            # cores
SL = S // NC      # 256 query rows per core
SC = SL // P      # 2
HC = H // P       # 6
DC = D // P       # 2
IC = I // P       # 9
KC = S // P       # 16 key chunks
NSL = S // 512    # 4 512-wide slices
EPS = 1e-6
C_SHIFT = 30.0    # exp(s - C_SHIFT): keeps unnormalized softmax in fp32 range

F32 = mybir.dt.float32
F32R = mybir.dt.float32r
BF16 = mybir.dt.bfloat16
MUL = mybir.AluOpType.mult
AF = mybir.ActivationFunctionType

_CACHED = {}


def _build():
    nc = bacc.Bacc("TRN2", target_bir_lowering=False, debug=False, num_devices=NC)

    # ---- DRAM I/O ----
    ht = nc.dram_tensor("ht", [H, S], F32R, kind="ExternalInput").ap()
    hq = nc.dram_tensor("hq", [H, SL], F32R, kind="ExternalInput").ap()
    cost = nc.dram_tensor("cost", [D, S], F32, kind="ExternalInput").ap()
    sint = nc.dram_tensor("sint", [D, S], F32, kind="ExternalInput").ap()
    cosq = nc.dram_tensor("cosq", [D, SL], F32, kind="ExternalInput").ap()
    sinq = nc.dram_tensor("sinq", [D, SL], F32, kind="ExternalInput").ap()
    wqt = nc.dram_tensor("wqt", [H, H], F32R, kind="ExternalInput").ap()
    wkt = nc.dram_tensor("wkt", [H, D], F32R, kind="ExternalInput").ap()
    wvt = nc.dram_tensor("wvt", [H, D], F32R, kind="ExternalInput").ap()
    wot = nc.dram_tensor("wot", [H, H], BF16, kind="ExternalInput").ap()
    wgt = nc.dram_tensor("wgt", [H, I], BF16, kind="ExternalInput").ap()
    wut = nc.dram_tensor("wut", [H, I], BF16, kind="ExternalInput").ap()
    wdt = nc.dram_tensor("wdt", [I, H], BF16, kind="ExternalInput").ap()
    qw1 = nc.dram_tensor("qw1", [P, DC], F32, kind="ExternalInput").ap()   # 1+q_norm_w
    kw1 = nc.dram_tensor("kw1", [P, DC], F32, kind="ExternalInput").ap()   # 1+k_norm_w
    wai = nc.dram_tensor("wai", [P, HC], BF16, kind="ExternalInput").ap()  # (1+ln_post_attn)^-2
    wfi = nc.dram_tensor("wfi", [P, HC], BF16, kind="ExternalInput").ap()  # (1+ln_post_ffn)^-2
    outt = nc.dram_tensor("outt", [H, SL], F32, kind="ExternalOutput").ap()

    def cp(ap2d):  # [(c p), x] -> [p, c, x]
        return ap2d.rearrange("(c p) x -> p c x", p=P)

    def f32(ap):
        return ap.bitcast(F32)

    with tile.TileContext(nc) as tc:
        with (
            tc.tile_pool(name="persist", bufs=1) as pp,
            tc.tile_pool(name="t1", bufs=2) as t1,
            tc.tile_pool(name="pmm", bufs=3, space="PSUM") as pmm,
            tc.tile_pool(name="pst", bufs=1, space="PSUM") as pst,
        ):
            # ---- small constants ----
            ones = pp.tile([P, 1], F32, tag="ones")
            nc.vector.memset(ones[:], 1.0)
            ones_r = ones[:].bitcast(F32R)
            ones_b = pp.tile([P, 1], BF16, tag="ones_b")
            nc.vector.memset(ones_b[:], 1.0)
            ones2b = pp.tile([P, 2], BF16, tag="ones2b")
            nc.vector.memset(ones2b[:], 1.0)
            eps128 = pp.tile([P, 1], F32, tag="eps128")
            nc.vector.memset(eps128[:], EPS)
            biasC = pp.tile([P, 1], F32, tag="biasC")
            nc.vector.memset(biasC[:], -C_SHIFT)
            qw1_sb = pp.tile([P, DC], F32, tag="qw1")
            nc.sync.dma_start(qw1_sb[:], qw1)
            kw1_sb = pp.tile([P, DC], F32, tag="kw1")
            nc.sync.dma_start(kw1_sb[:], kw1)
            wai_sb = pp.tile([P, HC], BF16, tag="wai")
            nc.sync.dma_start(wai_sb[:], wai)
            wfi_sb = pp.tile([P, HC], BF16, tag="wfi")
            nc.sync.dma_start(wfi_sb[:], wfi)

            # ---- startup-critical loads ----
            hq_sb = pp.tile([P, HC, SL], F32R, tag="hq")
            nc.sync.dma_start(hq_sb[:], cp(hq))
            es = ExitStack()
            wp1 = es.enter_context(tc.tile_pool(name="wp1", bufs=1))
            wk_sb = wp1.tile([P, HC, D], F32R, tag="wk")
            nc.scalar.dma_start(wk_sb[:], cp(wkt))
            wv_sb = wp1.tile([P, HC, D], F32R, tag="wv")
            nc.scalar.dma_start(wv_sb[:], cp(wvt))

            # persistent activations
            qt_f = pp.tile([P, HC, SL], F32R, tag="qtf")
            kt_sb = pp.tile([P, DC, S], F32R, tag="ktf")
            v_sb = pp.tile([P, KC, D], BF16, tag="v")
            at_f = pp.tile([P, DC, NH, SL], BF16, tag="atf")
            h2 = pp.tile([P, HC, SL], F32, tag="h2")
            h2n = pp.tile([P, HC, SL], BF16, tag="h2n")
            rin_col = pp.tile([P, KC], F32, tag="rin")
            ck_col = pp.tile([P, KC], F32, tag="ck")

            # =====================================================
            # K + stats over full S, streamed in 512-col slices
            # =====================================================
            esl = ExitStack()
            slp = esl.enter_context(tc.tile_pool(name="slp", bufs=2))
            psc = esl.enter_context(tc.tile_pool(name="psc", bufs=1, space="PSUM"))
            iss_t = psc.tile([P, KC, 2], F32, tag="ips", name="iss")
            ck_t = psc.tile([P, KC, 2], F32, tag="cps", name="cks")
            iss_ps = iss_t[:]
            ck_ps = ck_t[:]

            for sl in range(NSL):
                sls = slice(sl * 512, (sl + 1) * 512)
                hsl = slp.tile([P, HC, 512], F32R, tag="hsl", name=f"hsl{sl}")
                nc.sync.dma_start(hsl[:], cp(ht)[:, :, sls])
                cs = slp.tile([P, DC, 512], F32, tag="cs", name=f"cs{sl}")
                nc.scalar.dma_start(cs[:], cp(cost)[:, :, sls])
                sn = slp.tile([P, DC, 512], F32, tag="sn", name=f"sn{sl}")
                nc.scalar.dma_start(sn[:], cp(sint)[:, :, sls])

                # K projection for this slice
                pk = [psc.tile([P, 512], F32, tag="pk", name=f"pk{sl}_{d_}", bufs=2)
                      for d_ in range(DC)]
                for d in range(DC):
                    for kc in range(HC):
                        nc.tensor.matmul(
                            pk[d][:],
                            wk_sb[:, kc, d * P:(d + 1) * P],
                            hsl[:, kc, :],
                            start=(kc == 0), stop=(kc == HC - 1),
                        )
                # k-norm stats (column orientation)
                ksq = []
                for d in range(DC):
                    sq = t1.tile([P, 512], BF16, tag="ksq", name=f"ksq{sl}_{d}")
                    nc.scalar.activation(sq[:], pk[d][:], AF.Square)
                    ksq.append(sq)
                for c4 in range(4):
                    ksc = 4 * sl + c4
                    for d in range(DC):
                        nc.tensor.matmul(
                            ck_ps[:, ksc, :],
                            ksq[d][:, c4 * P:(c4 + 1) * P],
                            ones2b[:],
                            start=(d == 0), stop=(d == DC - 1),
                        )
                # rope -> kt
                t0 = t1.tile([P, 512], F32, tag="rA", name=f"krA{sl}")
                tb = t1.tile([P, 512], F32, tag="rB", name=f"krB{sl}")
                nc.vector.scalar_tensor_tensor(
                    t0[:], pk[0][:], kw1_sb[:, 0:1], cs[:, 0, :], MUL, MUL)
                nc.vector.scalar_tensor_tensor(
                    tb[:], pk[1][:], kw1_sb[:, 1:2], sn[:, 0, :], MUL, MUL)
                nc.vector.tensor_sub(kt_sb[:, 0, sls], t0[:], tb[:])
                t2 = t1.tile([P, 512], F32, tag="rA", name=f"krC{sl}")
                t3 = t1.tile([P, 512], F32, tag="rB", name=f"krD{sl}")
                nc.vector.scalar_tensor_tensor(
                    t2[:], pk[1][:], kw1_sb[:, 1:2], cs[:, 1, :], MUL, MUL)
                nc.vector.scalar_tensor_tensor(
                    t3[:], pk[0][:], kw1_sb[:, 0:1], sn[:, 1, :], MUL, MUL)
                nc.vector.tensor_add(kt_sb[:, 1, sls], t2[:], t3[:])

                # input sumsq (column orientation) for V scale
                isq_t = []
                for kc in range(HC):
                    isq = t1.tile([P, 512], BF16, tag="isq", name=f"isq{sl}_{kc}",
                                  bufs=HC)
                    nc.vector.tensor_mul(isq[:], f32(hsl[:, kc, :]),
                                         f32(hsl[:, kc, :]))
                    isq_t.append(isq)
                for c4 in range(4):
                    for kc in range(HC):
                        nc.tensor.matmul(
                            iss_ps[:, 4 * sl + c4, :],
                            isq_t[kc][:, c4 * P:(c4 + 1) * P],
                            ones2b[:],
                            start=(kc == 0), stop=(kc == HC - 1),
                        )

                # V projection for this slice (raw; rin applied afterwards)
                for c4 in range(4):
                    sc = 4 * sl + c4
                    pv = pmm.tile([P, D], F32, tag="mm", name=f"pv{sc}")
                    for kc in range(HC):
                        nc.tensor.matmul(
                            pv[:],
                            hsl[:, kc, c4 * P:(c4 + 1) * P],
                            wv_sb[:, kc, :],
                            start=(kc == 0), stop=(kc == HC - 1),
                        )
                    nc.scalar.copy(v_sb[:, sc, :], pv[:])

            # rstd columns: rin [P, KC], ck [P, KC]
            nc.scalar.activation(
                rin_col[:],
                iss_ps[:, :, 0:1].rearrange("p a b -> p (a b)"),
                AF.Sqrt, bias=eps128[:], scale=1.0 / H)
            nc.vector.reciprocal_approx_fast(rin_col[:], rin_col[:])
            nc.scalar.activation(
                ck_col[:],
                ck_ps[:, :, 0:1].rearrange("p a b -> p (a b)"),
                AF.Sqrt, bias=eps128[:], scale=1.0 / D)
            nc.vector.reciprocal_approx_fast(ck_col[:], ck_col[:])

            esl.close()  # free slice ring + slice psum

            # =====================================================
            # Q projection + q-norm + rope
            # =====================================================
            cosq_sb = pp.tile([P, DC, SL], F32, tag="cosq")
            nc.sync.dma_start(cosq_sb[:], cp(cosq))
            sinq_sb = pp.tile([P, DC, SL], F32, tag="sinq")
            nc.sync.dma_start(sinq_sb[:], cp(sinq))
            wq_sb = wp1.tile([P, HC, H], F32R, tag="wq")
            nc.sync.dma_start(wq_sb[:], cp(wqt))
            wo_sb = pp.tile([P, HC, H], BF16, tag="wo")
            nc.sync.dma_start(wo_sb[:], cp(wot))
            wg_sb = pp.tile([P, HC, I], BF16, tag="wg")
            nc.sync.dma_start(wg_sb[:], cp(wgt))
            wu_sb = pp.tile([P, HC, I], BF16, tag="wu")
            nc.sync.dma_start(wu_sb[:], cp(wut))
            wd_sb = pp.tile([P, IC, H], BF16, tag="wd")
            nc.sync.dma_start(wd_sb[:], cp(wdt))

            for h in range(NH):
                pq = [pmm.tile([P, SL], F32, tag="mm", name=f"pq{h}_{d_}")
                      for d_ in range(DC)]
                for d in range(DC):
                    oc = 2 * h + d
                    for kc in range(HC):
                        nc.tensor.matmul(
                            pq[d][:],
                            wq_sb[:, kc, oc * P:(oc + 1) * P],
                            hq_sb[:, kc, :],
                            start=(kc == 0), stop=(kc == HC - 1),
                        )
                qss = pst.tile([1, SL], F32, tag="st1", name=f"qss{h}")
                for d in range(DC):
                    sq = t1.tile([P, SL], F32R, tag="sq", name=f"qsq{h}_{d}")
                    nc.scalar.activation(sq[:], pq[d][:], AF.Square)
                    nc.tensor.matmul(qss[:], ones_r, sq[:],
                                     start=(d == 0), stop=(d == DC - 1))
                cq_row = t1.tile([1, SL], F32, tag="row", name=f"cqr{h}")
                nc.scalar.activation(cq_row[:], qss[:], AF.Sqrt,
                                     bias=eps128[0:1, :], scale=1.0 / D)
                nc.vector.reciprocal_approx_fast(cq_row[:], cq_row[:])
                cq_b = t1.tile([P, SL], F32, tag="bcast", name=f"cqb{h}")
                nc.gpsimd.partition_broadcast(cq_b[:], cq_row[:], channels=P)
                t0 = t1.tile([P, SL], F32, tag="rA", name=f"rA{h}")
                tb = t1.tile([P, SL], F32, tag="rB", name=f"rB{h}")
                nc.vector.scalar_tensor_tensor(
                    t0[:], pq[0][:], qw1_sb[:, 0:1], cosq_sb[:, 0, :], MUL, MUL)
                nc.vector.scalar_tensor_tensor(
                    tb[:], pq[1][:], qw1_sb[:, 1:2], sinq_sb[:, 0, :], MUL, MUL)
                nc.vector.tensor_sub(t0[:], t0[:], tb[:])
                nc.vector.tensor_mul(qt_f[:, 2 * h, :], t0[:], cq_b[:])
                t2 = t1.tile([P, SL], F32, tag="rA", name=f"rC{h}")
                t3 = t1.tile([P, SL], F32, tag="rB", name=f"rD{h}")
                nc.vector.scalar_tensor_tensor(
                    t2[:], pq[1][:], qw1_sb[:, 1:2], cosq_sb[:, 1, :], MUL, MUL)
                nc.vector.scalar_tensor_tensor(
                    t3[:], pq[0][:], qw1_sb[:, 0:1], sinq_sb[:, 1, :], MUL, MUL)
                nc.vector.tensor_add(t2[:], t2[:], t3[:])
                nc.vector.tensor_mul(qt_f[:, 2 * h + 1, :], t2[:], cq_b[:])

            es.close()  # free wq/wk/wv

            # =====================================================
            # attention: scoresT -> exp(scale=ck) -> den -> A^T V
            # =====================================================
            with tc.tile_pool(name="t2", bufs=2) as t2p:
                u3 = t2p.tile([P, NH, KC, SL], BF16, tag="u3", name="u3", bufs=1)
                den_b = t2p.tile([P, NH, SL], F32, tag="denb", name="den_b", bufs=1)
                esa = ExitStack()
                psa = esa.enter_context(
                    tc.tile_pool(name="psa", bufs=1, space="PSUM"))
                # heads 0+1 paired into 512-wide matmuls; head 2 narrow
                for ksc in range(KC):
                    ps2 = psa.tile([P, 2, SL], F32, tag="mm2", name=f"ps2_{ksc}",
                                   bufs=2)
                    for d in range(DC):
                        nc.tensor.matmul(
                            ps2[:],
                            kt_sb[:, d, ksc * P:(ksc + 1) * P],
                            qt_f[:, d:3 + d:2, :],
                            start=(d == 0), stop=(d == DC - 1),
                        )
                    nc.scalar.activation(u3[:, 0:2, ksc, :], ps2[:], AF.Exp,
                                         bias=biasC[:],
                                         scale=ck_col[:, ksc:ksc + 1])
                    ps_ = pmm.tile([P, SL], F32, tag="mm", name=f"ps{ksc}")
                    for d in range(DC):
                        nc.tensor.matmul(
                            ps_[:],
                            kt_sb[:, d, ksc * P:(ksc + 1) * P],
                            qt_f[:, 4 + d, :],
                            start=(d == 0), stop=(d == DC - 1),
                        )
                    nc.scalar.activation(u3[:, 2, ksc, :], ps_[:], AF.Exp,
                                         bias=biasC[:],
                                         scale=ck_col[:, ksc:ksc + 1])

                # apply rin to V (off the exp critical path)
                for sc in range(KC):
                    nc.scalar.mul(v_sb[:, sc, :], v_sb[:, sc, :],
                                  rin_col[:, sc:sc + 1])

                den2 = psa.tile([1, 2, SL], F32, tag="dn2", name="den01")
                for ksc in range(KC):
                    nc.tensor.matmul(
                        den2[:], ones_b[:], u3[:, 0:2, ksc, :],
                        start=(ksc == 0), stop=(ksc == KC - 1))
                den2_row = t1.tile([1, 2, SL], F32, tag="row2", name="denr01")
                nc.scalar.copy(den2_row[:], den2[:])
                nc.vector.reciprocal_approx_fast(
                    den2_row[:].rearrange("o a s -> o (a s)"),
                    den2_row[:].rearrange("o a s -> o (a s)"))
                for h in range(2):
                    nc.gpsimd.partition_broadcast(den_b[:, h, :],
                                                  den2_row[:, h, :], channels=P)
                den = pst.tile([1, SL], F32, tag="st1", name="den2")
                for ksc in range(KC):
                    nc.tensor.matmul(den[:], ones_b[:], u3[:, 2, ksc, :],
                                     start=(ksc == 0), stop=(ksc == KC - 1))
                den_row = t1.tile([1, SL], F32, tag="row", name="denr2")
                nc.scalar.copy(den_row[:], den[:])
                nc.vector.reciprocal_approx_fast(den_row[:], den_row[:])
                nc.gpsimd.partition_broadcast(den_b[:, 2, :], den_row[:],
                                              channels=P)

                for d in range(DC):
                    pn2 = psa.tile([P, 2, SL], F32, tag="mm2", name=f"pn2_{d}",
                                   bufs=2)
                    for ksc in range(KC):
                        nc.tensor.matmul(
                            pn2[:],
                            v_sb[:, ksc, d * P:(d + 1) * P],
                            u3[:, 0:2, ksc, :],
                            start=(ksc == 0), stop=(ksc == KC - 1),
                        )
                    nc.vector.tensor_mul(at_f[:, d, 0:2, :], pn2[:],
                                         den_b[:, 0:2, :])
                    pn = pmm.tile([P, SL], F32, tag="mm", name=f"pn{d}")
                    for ksc in range(KC):
                        nc.tensor.matmul(
                            pn[:],
                            v_sb[:, ksc, d * P:(d + 1) * P],
                            u3[:, 2, ksc, :],
                            start=(ksc == 0), stop=(ksc == KC - 1),
                        )
                    nc.vector.tensor_mul(at_f[:, d, 2, :], pn[:],
                                         den_b[:, 2, :])
                esa.close()
                at_v = at_f[:].rearrange("p d h s -> p (d h) s")

                # =====================================================
                # wo projection + post-attn rmsnorm + residual
                # =====================================================
                with tc.tile_pool(name="pho", bufs=1, space="PSUM") as pho:
                    ppo2 = [pho.tile([P, 2, SL], F32, tag=f"po{j}", name=f"pp{j}")
                            for j in range(HC // 2)]
                    ppo = [ppo2[j][:, i, :] for j in range(HC // 2) for i in range(2)]
                    pss = pst.tile([1, SL], F32, tag="st1", name="pss")
                    for hc in range(HC):
                        for oc in range(HC):
                            nc.tensor.matmul(
                                ppo[hc],
                                wo_sb[:, oc, hc * P:(hc + 1) * P],
                                at_v[:, oc, :],
                                start=(oc == 0), stop=(oc == HC - 1),
                            )
                        sq = t1.tile([P, SL], BF16, tag="sqb", name=f"psq{hc}")
                        nc.scalar.activation(sq[:], ppo[hc], AF.Square)
                        nc.tensor.matmul(pss[:], wai_sb[:, hc:hc + 1], sq[:],
                                         start=(hc == 0), stop=(hc == HC - 1))
                    ra_row = t1.tile([1, SL], F32, tag="row", name="ra_row")
                    nc.scalar.activation(ra_row[:], pss[:], AF.Sqrt,
                                         bias=eps128[0:1, :], scale=1.0 / H)
                    nc.vector.reciprocal_approx_fast(ra_row[:], ra_row[:])
                    ra_b = t1.tile([P, SL], F32, tag="bcast", name="ra_b")
                    nc.gpsimd.partition_broadcast(ra_b[:], ra_row[:], channels=P)
                    for hc in range(HC):
                        tm = t1.tile([P, SL], F32, tag="htmp", name=f"hm{hc}")
                        nc.vector.tensor_mul(tm[:], ppo[hc], ra_b[:])
                        nc.vector.tensor_add(h2[:, hc, :], tm[:],
                                             f32(hq_sb[:, hc, :]))

                # =====================================================
                # pre-FFN rmsnorm
                # =====================================================
                fss = pst.tile([1, SL], F32, tag="st1", name="fss")
                for hc in range(HC):
                    sq = t1.tile([P, SL], BF16, tag="sqb", name=f"fsq{hc}")
                    nc.vector.tensor_mul(sq[:], h2[:, hc, :], h2[:, hc, :])
                    nc.tensor.matmul(fss[:], ones_b[:], sq[:],
                                     start=(hc == 0), stop=(hc == HC - 1))
                r2_row = t1.tile([1, SL], F32, tag="row", name="r2_row")
                nc.scalar.activation(r2_row[:], fss[:], AF.Sqrt,
                                     bias=eps128[0:1, :], scale=1.0 / H)
                nc.vector.reciprocal_approx_fast(r2_row[:], r2_row[:])
                r2_b = t1.tile([P, SL], F32, tag="bcast", name="r2_b")
                nc.gpsimd.partition_broadcast(r2_b[:], r2_row[:], channels=P)
                for hc in range(HC):
                    nc.vector.tensor_mul(h2n[:, hc, :], h2[:, hc, :], r2_b[:])

                # =====================================================
                # MLP: gate/up -> gelu_tanh * up -> down + post-ffn norm
                # =====================================================
                gall = t2p.tile([P, IC, SL], BF16, tag="gall", name="gall", bufs=1)
                act = t2p.tile([P, IC, SL], BF16, tag="act", name="act", bufs=1)
                for ic in range(IC):
                    pg = pmm.tile([P, SL], F32, tag="mm", name=f"pg{ic}")
                    for kc in range(HC):
                        nc.tensor.matmul(pg[:], wg_sb[:, kc, ic * P:(ic + 1) * P],
                                         h2n[:, kc, :],
                                         start=(kc == 0), stop=(kc == HC - 1))
                    nc.scalar.activation(gall[:, ic, :], pg[:], AF.Gelu_apprx_tanh)
                    pu = pmm.tile([P, SL], F32, tag="mm", name=f"pu{ic}")
                    for kc in range(HC):
                        nc.tensor.matmul(pu[:], wu_sb[:, kc, ic * P:(ic + 1) * P],
                                         h2n[:, kc, :],
                                         start=(kc == 0), stop=(kc == HC - 1))
                    nc.vector.tensor_mul(act[:, ic, :], gall[:, ic, :], pu[:])

                with tc.tile_pool(name="phd", bufs=1, space="PSUM") as phd:
                    pm2 = [phd.tile([P, 2, SL], F32, tag=f"md{j}", name=f"pm{j}")
                           for j in range(HC // 2)]
                    pm = [pm2[j][:, i, :] for j in range(HC // 2) for i in range(2)]
                    mss = pst.tile([1, SL], F32, tag="st1", name="mss")
                    for hc in range(HC):
                        for ic in range(IC):
                            nc.tensor.matmul(pm[hc],
                                             wd_sb[:, ic, hc * P:(hc + 1) * P],
                                             act[:, ic, :],
                                             start=(ic == 0), stop=(ic == IC - 1))
                        sq = t1.tile([P, SL], BF16, tag="sqb", name=f"msq{hc}")
                        nc.scalar.activation(sq[:], pm[hc], AF.Square)
                        nc.tensor.matmul(mss[:], wfi_sb[:, hc:hc + 1], sq[:],
                                         start=(hc == 0), stop=(hc == HC - 1))
                    r3_row = t1.tile([1, SL], F32, tag="row", name="r3_row")
                    nc.scalar.activation(r3_row[:], mss[:], AF.Sqrt,
                                         bias=eps128[0:1, :], scale=1.0 / H)
                    nc.vector.reciprocal_approx_fast(r3_row[:], r3_row[:])
                    r3_b = t1.tile([P, SL], F32, tag="bcast", name="r3_b")
                    nc.gpsimd.partition_broadcast(r3_b[:], r3_row[:], channels=P)
                    for hc in range(HC):
                        tm = t1.tile([P, SL], F32, tag="htmp", name=f"om{hc}")
                        nc.vector.tensor_mul(tm[:], pm[hc], r3_b[:])
                        out_c = t1.tile([P, SL], F32, tag="outc", name=f"oc{hc}",
                                        bufs=3)
                        nc.vector.tensor_add(out_c[:], tm[:], h2[:, hc, :])
                        nc.sync.dma_start(cp(outt)[:, hc, :], out_c[:])

    nc.compile()
    return nc


def _get_nc():
    if "nc" not in _CACHED:
        _CACHED["nc"] = _build()
    return _CACHED["nc"]


def _prep_inputs(hidden_states, cos, sin, wq, wk, wv, wo, q_norm_w, k_norm_w,
                 ln_in_w, ln_post_attn_w, ln_pre_ffn_w, ln_post_ffn_w,
                 wg, wu, wd):
    f = np.float32
    bf = ml_dtypes.bfloat16
    ct = np.ascontiguousarray

    hid = np.asarray(hidden_states, f)[0]            # [S, H]
    hT = ct(hid.T)                                   # [H, S]
    cosT = ct(np.asarray(cos, f)[0, 0].T)            # [D, S]
    sinT = ct(np.asarray(sin, f)[0, 0].T)

    g_in = 1.0 + np.asarray(ln_in_w, f)
    g_ffn = 1.0 + np.asarray(ln_pre_ffn_w, f)
    g_att = 1.0 + np.asarray(ln_post_attn_w, f)
    g_out = 1.0 + np.asarray(ln_post_ffn_w, f)

    shared = {
        "ht": hT,
        "cost": cosT,
        "sint": sinT,
        "wqt": ct((np.asarray(wq, f) * g_in[None, :]).T),
        "wkt": ct((np.asarray(wk, f) * g_in[None, :]).T),
        "wvt": ct((np.asarray(wv, f) * g_in[None, :]).T),
        "wot": ct((np.asarray(wo, f).T * g_att[None, :])
                  .reshape(NH, DC, P, H)
                  .transpose(1, 0, 2, 3).reshape(H, H).astype(bf)),
        "wgt": ct(((np.asarray(wg, f) * g_ffn[None, :]).T).astype(bf)),
        "wut": ct(((np.asarray(wu, f) * g_ffn[None, :]).T).astype(bf)),
        "wdt": ct((np.asarray(wd, f).T * g_out[None, :]).astype(bf)),
        "qw1": ct((1.0 + np.asarray(q_norm_w, f)).reshape(DC, P).T),
        "kw1": ct((1.0 + np.asarray(k_norm_w, f)).reshape(DC, P).T),
        "wai": ct((g_att ** -2.0).reshape(HC, P).T.astype(bf)),
        "wfi": ct((g_out ** -2.0).reshape(HC, P).T.astype(bf)),
    }
    in_maps = []
    for c in range(NC):
        cols = slice(c * SL, (c + 1) * SL)
        m = dict(shared)
        m["hq"] = ct(hT[:, cols])
        m["cosq"] = ct(cosT[:, cols])
        m["sinq"] = ct(sinT[:, cols])
        in_maps.append(m)
    return in_maps


def run(trace=False, tmpdir=None, **inputs):
    """Build (cached), run on 8 cores, reassemble. Returns (output, results)."""
    nc = _get_nc()
    in_maps = _prep_inputs(
        inputs["hidden_states"], inputs["cos"], inputs["sin"],
        inputs["wq"], inputs["wk"], inputs["wv"], inputs["wo"],
        inputs["q_norm_w"], inputs["k_norm_w"],
        inputs["ln_in_w"], inputs["ln_post_attn_w"],
        inputs["ln_pre_ffn_w"], inputs["ln_post_ffn_w"],
        inputs["wg"], inputs["wu"], inputs["wd"],
    )
    res = run_bass_kernel_spmd(nc, in_maps, list(range(NC)),
                               trace=trace, tmpdir=tmpdir)
    out = np.empty((S, H), np.float32)
    for c in range(NC):
        out[c * SL:(c + 1) * SL, :] = res.results[c]["outt"].T
    return out[None], res


def kernel(**inputs):
    out, _ = run(trace=False, **inputs)
    return out
